# revision 16
# baseline (speedup 1.0000x reference)
"""3-layer GAT on 8 Trainium2 NeuronCores (Bass/Tile).

Sharding: 2D graph partition. Pair q = cores {2q, 2q+1} aggregates the dst
nodes of strips [q*6250,(q+1)*6250) and [25000+q*6250, 25000+(q+1)*6250);
even cores take edges with src < 25000, odd cores the rest. Node ownership:
core 2k owns rows [k*6250,(k+1)*6250), core 2k+1 owns [25000+k*6250, ...).

Per layer: each core projects its own rows (feat|el|er via an augmented
weight matrix) into a local gather table, then runs the edge phase in two
passes: pass A covers edges whose source is one of the core's own rows and
gathers from the local table while the quad AllGather of the full src-half
table is still in flight; pass B covers the remaining edges and gathers
from the AllGathered table. Edges are packed into 128-wide tiles grouped
by pairs of 128-dst blocks (a tile may straddle the two blocks; the
host-precomputed one-hot masks select membership). Per-edge er comes from
a transposed one-hot matmul against SBUF-resident per-block er rows (no
second gather). Messages are accumulated per dst block by one-hot-mask
matmuls into PSUM; pass B adds pass A's partial sums back in. Partial sums
are pairwise ReduceScattered in two halves (the first fires mid-pass-B),
then a batched divide/bias/ELU (head-mean on the last layer) produces the
output rows, fused with the next layer's projection.
"""

import numpy as np
import ml_dtypes

N = 50000
E = 800000
F = 128                  # input feats and hidden width (4 heads x 32)
H = 4
D = 32
NEG = 0.2
NCORE = 8
NPC = 6250               # nodes owned per core
OWN = 6272               # 49*128, padded own rows
OWNBLK = 49
PAIR = 12544             # 98*128 dst slots per pair
NBLK = 98
NGRP = 49                # pair-groups of 2 blocks
HALF = 25088             # 4*OWN rows per src-half table
TROWS = 25216            # HALF + 128 (dummy row at HALF)
DUMMY = HALF
TCOLS = 256              # bf16 cols: feat(128) | el(4) | pad
CHUNK = 28               # max tiles per dma_gather call
GROUP = 8                # tiles per vector-op batch
EPS = 1e-30

# pair-group processing order: finish half-1 blocks ({0..23} u {49..72})
# first so the first ReduceScatter can fire mid-pass.
GORDER = list(range(0, 12)) + list(range(24, 37)) + \
         list(range(12, 24)) + list(range(37, 49))
H1_BLOCKS = set(range(0, 24)) | set(range(49, 73))

# block -> partial-row-block permutation: [A1(24) B1(24) A2(25) B2(25)]
ROW_OF = np.empty(NBLK, np.int64)
ROW_OF[0:24] = np.arange(24)
ROW_OF[49:73] = 24 + np.arange(24)
ROW_OF[24:49] = 48 + np.arange(25)
ROW_OF[73:98] = 73 + np.arange(25)

_cache = {}


def _schedule(cnt):
    """Core-uniform tile/mask schedule for one pass.

    cnt: [NCORE, NBLK] per-core per-block edge counts.
    Returns dict with T, nmask, chunks, tiles (per tile: list of
    (mslot, block, er_first, er_last, sc_first, sc_last)).
    """
    n0 = cnt[:, 0::2]                      # [NCORE, NGRP]
    n1 = cnt[:, 1::2]
    TP = np.maximum(1, np.ceil((n0 + n1).max(axis=0) / 128).astype(np.int64))

    base_tile = {}
    acc = 0
    for g in GORDER:
        base_tile[g] = acc
        acc += int(TP[g])
    T = acc

    tiles = []            # per tile: list of [mslot, block]
    tile_group = []
    # which (tile-in-group, block-parity) pairs are needed on any core; ensure
    # every block gets at least one occurrence (tile 0 fallback)
    need = {}
    for g in GORDER:
        for i in range(int(TP[g])):
            need[(g, i, 0)] = bool((n0[:, g] > 128 * i).any())
            need[(g, i, 1)] = bool(
                ((n0[:, g] < 128 * (i + 1)) &
                 (n0[:, g] + n1[:, g] > 128 * i)).any())
        if not any(need[(g, i, 0)] for i in range(int(TP[g]))):
            need[(g, 0, 0)] = True
        if not any(need[(g, i, 1)] for i in range(int(TP[g]))):
            need[(g, 0, 1)] = True
    mslot = 0
    for g in GORDER:
        for i in range(int(TP[g])):
            ml = []
            if need[(g, i, 0)]:
                ml.append([mslot, 2 * g])
                mslot += 1
            if need[(g, i, 1)]:
                ml.append([mslot, 2 * g + 1])
                mslot += 1
            assert ml
            tiles.append(ml)
            tile_group.append(g)
    nmask = mslot

    # per-block first/last occurrence
    occ = {}
    for ti, ml in enumerate(tiles):
        for m in ml:
            occ.setdefault(m[1], []).append((ti, m[0]))
    first = {b: o[0] for b, o in occ.items()}
    last = {b: o[-1] for b, o in occ.items()}
    sched_tiles = []
    for ti, ml in enumerate(tiles):
        entry = []
        for k, (ms, b) in enumerate(ml):
            entry.append((ms, b,
                          k == 0, k == len(ml) - 1,
                          first[b] == (ti, ms), last[b] == (ti, ms)))
        sched_tiles.append(entry)

    # chunks aligned to pair-group boundaries, up to CHUNK tiles
    chunks = []
    t0 = 0
    ti = 0
    for g in GORDER:
        ti += int(TP[g])
        nxt = None
        gi = GORDER.index(g)
        if gi + 1 < len(GORDER):
            nxt = int(TP[GORDER[gi + 1]])
        if nxt is None or ti - t0 + nxt > CHUNK:
            m0 = min(m[0] for m in sched_tiles[t0]) if sched_tiles[t0] else 0
            mend = max(m[0] for m in sched_tiles[ti - 1]) + 1
            chunks.append((t0, ti - t0, m0, mend - m0))
            t0 = ti
    assert t0 == T
    return dict(T=T, nmask=nmask, chunks=chunks, tiles=sched_tiles,
                tile_group=tile_group, base_tile=base_tile, TP=TP)


def _wrap16(a):
    # value i of each 128-group at [i%16, i//16], replicated per 16 rows
    t = a.reshape(-1, 128)                     # [T, 128]
    w = t.reshape(t.shape[0], 8, 16)           # [T, 8, 16]
    w = w.transpose(2, 0, 1).reshape(16, -1)   # [16, T*8]
    return np.tile(w, (8, 1)).astype(np.int16)  # [128, T*8]


def _core_pass_arrays(sched, rloc_e, rows_e, pad_row):
    """Build idx + mask streams for one (core, pass).

    rloc_e: pair-local dst row per edge; rows_e: gather-table row per edge.
    """
    T, nmask = sched["T"], sched["nmask"]
    base_tile = sched["base_tile"]
    # group rank of each edge
    grank_of = np.empty(NGRP, np.int64)
    for r, g in enumerate(GORDER):
        grank_of[g] = r
    pg = rloc_e // 256
    gr = grank_of[pg]
    order = np.lexsort((rloc_e, gr))
    rloc_s = rloc_e[order]
    rows_s = rows_e[order]
    gr_s = gr[order]
    # position within group
    starts = np.searchsorted(gr_s, np.arange(len(GORDER)))
    pos_in_group = np.arange(len(gr_s)) - starts[gr_s]
    base128 = np.array([base_tile[GORDER[r]] * 128
                        for r in range(len(GORDER))], np.int64)
    s_glob = base128[gr_s] + pos_in_group

    idx = np.full(T * 128, pad_row, np.int64)
    idx[s_glob] = rows_s

    # mask slot lookup per (tile, block-parity)
    mslot_of = np.full((T, 2), -1, np.int64)
    for ti, ml in enumerate(sched["tiles"]):
        g = sched["tile_group"][ti]
        for (ms, b, *_fl) in ml:
            mslot_of[ti, b - 2 * g] = ms
    ti_e = s_glob // 128
    e_e = s_glob % 128
    b_e = rloc_s // 128
    s128_e = rloc_s % 128
    par = b_e - 2 * np.array(sched["tile_group"])[ti_e]
    ms_e = mslot_of[ti_e, par]
    assert (ms_e >= 0).all()

    smatw = np.zeros((128, nmask * 128), ml_dtypes.bfloat16)
    smatw[e_e, ms_e * 128 + s128_e] = 1
    smatTw = np.zeros((128, nmask * 128), ml_dtypes.bfloat16)
    smatTw[s128_e, ms_e * 128 + e_e] = 1
    return _wrap16(idx), smatw, smatTw


def _preprocess(src, dst):
    src = np.asarray(src).astype(np.int64)
    dst = np.asarray(dst).astype(np.int64)
    q = np.where(dst < 25000, dst // NPC, (dst - 25000) // NPC)
    s = (src >= 25000).astype(np.int64)
    core_of = 2 * q + s
    rloc = np.where(dst < 25000, dst - q * NPC, OWN + (dst - 25000 - q * NPC))
    ks = np.where(src < 25000, src // NPC, (src - 25000) // NPC)
    tloc = np.where(src < 25000, OWN * ks + src - ks * NPC,
                    OWN * ks + (src - 25000) - ks * NPC)
    own = ks == q
    ownrow = np.where(src < 25000, src - ks * NPC, src - 25000 - ks * NPC)
    blk = rloc // 128

    cntA = np.zeros((NCORE, NBLK), np.int64)
    cntB = np.zeros((NCORE, NBLK), np.int64)
    for c in range(NCORE):
        m = core_of == c
        cntA[c] = np.bincount(blk[m & own], minlength=NBLK)
        cntB[c] = np.bincount(blk[m & ~own], minlength=NBLK)
    schedA = _schedule(cntA)
    schedB = _schedule(cntB)

    cores = []
    for c in range(NCORE):
        m = core_of == c
        mA = m & own
        mB = m & ~own
        idxA, smA, smTA = _core_pass_arrays(schedA, rloc[mA], ownrow[mA], OWN)
        idxB, smB, smTB = _core_pass_arrays(schedB, rloc[mB], tloc[mB], DUMMY)
        cores.append(dict(idxA=idxA, smA=smA, smTA=smTA,
                          idxB=idxB, smB=smB, smTB=smTB))
    return cores, schedA, schedB


def _own_rows(c):
    k = c // 2
    if c % 2 == 0:
        return k * NPC, (k + 1) * NPC
    return 25000 + k * NPC, 25000 + (k + 1) * NPC


def _augment(W, al, ar):
    dout = W.shape[1] // H
    Wal = np.stack([W[:, h * dout:(h + 1) * dout] @ al[h] for h in range(H)], 1)
    War = np.stack([W[:, h * dout:(h + 1) * dout] @ ar[h] for h in range(H)], 1)
    return np.concatenate([W, Wal, War], 1).astype(np.float32)  # [128, 136]


def _build(schedA, schedB, consts, no_cc=False):
    import concourse.bass as bass
    import concourse.bacc as bacc
    import concourse.tile as tile
    from concourse import mybir
    from concourse.library_config import mlp

    f32 = mybir.dt.float32
    bf16 = mybir.dt.bfloat16
    i16 = mybir.dt.int16
    AF = mybir.ActivationFunctionType
    OP = mybir.AluOpType

    TA, TB = schedA["T"], schedB["T"]
    NMA, NMB = schedA["nmask"], schedB["nmask"]
    NM_MAX = max(max(nm for (_, _, _, nm) in schedA["chunks"]),
                 max(nm for (_, _, _, nm) in schedB["chunks"]))

    nc = bacc.Bacc(num_devices=NCORE)
    xT_in = nc.declare_dram_parameter("xT", [128, OWN], f32, isOutput=False)
    idxA_in = nc.declare_dram_parameter("idxA", [128, TA * 8], i16,
                                        isOutput=False)
    idxB_in = nc.declare_dram_parameter("idxB", [128, TB * 8], i16,
                                        isOutput=False)
    smA_in = nc.declare_dram_parameter("smA", [128, NMA * 128], bf16,
                                       isOutput=False)
    smTA_in = nc.declare_dram_parameter("smTA", [128, NMA * 128], bf16,
                                        isOutput=False)
    smB_in = nc.declare_dram_parameter("smB", [128, NMB * 128], bf16,
                                       isOutput=False)
    smTB_in = nc.declare_dram_parameter("smTB", [128, NMB * 128], bf16,
                                        isOutput=False)
    y_out = nc.declare_dram_parameter("y", [NPC, D], f32, isOutput=True)

    with tile.TileContext(nc) as tc:
        with tc.tile_pool(name="persist", bufs=1) as pp, \
             tc.tile_pool(name="dram", bufs=1, space="DRAM") as dp:
            nc.gpsimd.load_library(mlp)

            # ---- persistent SBUF state ----
            idxA_sb = pp.tile([128, TA * 8], i16)
            nc.sync.dma_start(out=idxA_sb[:], in_=idxA_in[:, :])
            idxB_sb = pp.tile([128, TB * 8], i16)
            nc.sync.dma_start(out=idxB_sb[:], in_=idxB_in[:, :])
            hT = pp.tile([128, OWN], f32)
            nc.sync.dma_start(out=hT[:], in_=xT_in[:, :])
            hT2 = pp.tile([128, OWN], f32)

            ident_h = nc.inline_tensor(np.eye(128, dtype=np.float32),
                                       name="ident")
            ident_sb = pp.tile([128, 128], f32)
            nc.sync.dma_start(out=ident_sb[:], in_=ident_h[:, :])

            waug_sb = []
            brep_sb = []
            for li in range(3):
                wh = nc.inline_tensor(consts[f"Waug{li}"], name=f"waug{li}")
                wt = pp.tile([128, 136], f32, name=f"waug_sb{li}")
                nc.sync.dma_start(out=wt[:], in_=wh[:, :])
                waug_sb.append(wt)
                bh = nc.inline_tensor(consts[f"brep{li}"], name=f"brep{li}")
                bt = pp.tile([128, consts[f"brep{li}"].shape[1]], f32,
                             name=f"brep_sb{li}")
                nc.sync.dma_start(out=bt[:], in_=bh[:, :])
                brep_sb.append(bt)

            dummy_h = nc.inline_tensor(consts["dummyrow"], name="dummyrow")

            # ---- DRAM scratch ----
            table = dp.tile([TROWS, TCOLS], bf16)
            er_tab = dp.tile([PAIR, 4], bf16)
            ag_own = dp.tile([OWN + 128, TCOLS], bf16)
            ag_er = dp.tile([OWN, 4], bf16)
            partialA = dp.tile([PAIR, 132], f32)
            partial = dp.tile([PAIR, 132], f32)
            own_sum = dp.tile([OWN, 132], f32)

            nc.sync.dma_start(out=table[DUMMY:DUMMY + 1, :], in_=dummy_h[:, :])
            nc.sync.dma_start(out=ag_own[OWN:OWN + 1, :], in_=dummy_h[:, :])

            groups_pair = [[2 * k, 2 * k + 1] for k in range(4)]
            groups_quad = [[0, 2, 4, 6], [1, 3, 5, 7]]

            def proj_phase(li, src_hT):
                with tc.tile_pool(name=f"prj{li}", bufs=3) as sp, \
                     tc.tile_pool(name=f"prjps{li}", bufs=3,
                                  space="PSUM") as ps:
                    # er-only mini-projection first so the small er AllGather
                    # launches early (pass A consumes it)
                    errow = sp.tile([128, OWNBLK, 4], bf16, name=f"errow{li}",
                                    tag="errow", bufs=1)
                    for t in range(OWNBLK):
                        pje = ps.tile([128, 4], f32, space="PSUM", tag="pje")
                        nc.tensor.matmul(pje[:],
                                         lhsT=src_hT[:, t * 128:(t + 1) * 128],
                                         rhs=waug_sb[li][:, 132:136],
                                         start=True, stop=True)
                        nc.scalar.activation(errow[:, t, :], pje[:], AF.Copy)
                    nc.sync.dma_start(
                        out=ag_er[:, :].rearrange("(t p) c -> p t c", p=128),
                        in_=errow[:])
                    if no_cc:
                        for rep in range(2):
                            nc.sync.dma_start(
                                out=er_tab[rep * OWN:(rep + 1) * OWN, :],
                                in_=ag_er[:, :])
                    else:
                        nc.gpsimd.collective_compute(
                            "AllGather", mybir.AluOpType.bypass,
                            replica_groups=groups_pair,
                            ins=[ag_er[:, :]], outs=[er_tab[:, :]])

                    tabrow = sp.tile([128, OWNBLK, TCOLS], bf16,
                                     name=f"tabrow{li}", tag="tabrow", bufs=1)
                    for t in range(OWNBLK):
                        pj = ps.tile([128, 132], f32, space="PSUM", tag="pj")
                        nc.tensor.matmul(pj[:],
                                         lhsT=src_hT[:, t * 128:(t + 1) * 128],
                                         rhs=waug_sb[li][:, 0:132], start=True,
                                         stop=True)
                        nc.scalar.activation(tabrow[:, t, 0:132], pj[:],
                                             AF.Copy)
                    nc.sync.dma_start(
                        out=ag_own[0:OWN, :]
                            .rearrange("(t p) c -> p t c", p=128),
                        in_=tabrow[:])
                if no_cc:
                    for rep in range(4):
                        nc.sync.dma_start(
                            out=table[rep * OWN:(rep + 1) * OWN, :],
                            in_=ag_own[0:OWN, :])
                else:
                    nc.gpsimd.collective_compute(
                        "AllGather", mybir.AluOpType.bypass,
                        replica_groups=groups_quad,
                        ins=[ag_own[0:OWN, :]], outs=[table[0:HALF, :]])

            proj_phase(0, hT)

            for li in range(3):
                last = li == 2

                with tc.tile_pool(name=f"gt{li}", bufs=4) as gp, \
                     tc.tile_pool(name=f"mk{li}", bufs=3) as mkp, \
                     tc.tile_pool(name=f"ms{li}", bufs=4) as mp, \
                     tc.tile_pool(name=f"ex{li}", bufs=4) as xp, \
                     tc.tile_pool(name=f"pb{li}", bufs=4) as pbp, \
                     tc.tile_pool(name=f"pa{li}", bufs=4) as pap, \
                     tc.tile_pool(name=f"sg{li}", bufs=4,
                                  space="PSUM") as sgps, \
                     tc.tile_pool(name=f"er{li}", bufs=3,
                                  space="PSUM") as erps:
                    er_sb = mp.tile([128, NBLK, 4], bf16, tag="er_sb", bufs=1,
                                    name=f"er_sb{li}")
                    nc.sync.dma_start(
                        out=er_sb[:],
                        in_=er_tab[:, :].rearrange("(t p) c -> p t c", p=128))

                    def edge_pass(sched, idx_sb, sm_in, smT_in, tab, passB):
                        seg_tiles = {}
                        pa_tiles = {}
                        h1rem = [len(H1_BLOCKS)]
                        state = {}

                        def emit_front(ci):
                            # gather + mask streams + er matmuls for chunk ci
                            (t0, nt, m0, nm) = sched["chunks"][ci]
                            g = gp.tile([128, CHUNK, TCOLS], bf16, tag="g")
                            nc.gpsimd.dma_gather(
                                out_ap=g[:, 0:nt, :], in_ap=tab[:, :],
                                idxs_ap=idx_sb[:, t0 * 8:(t0 + nt) * 8],
                                num_idxs=nt * 128, num_idxs_reg=nt * 128,
                                elem_size=TCOLS, single_packet=False)
                            sm = mkp.tile([128, NM_MAX * 128], bf16, tag="sm")
                            nc.sync.dma_start(
                                out=sm[:, 0:nm * 128],
                                in_=sm_in[:, m0 * 128:(m0 + nm) * 128])
                            smT = mkp.tile([128, NM_MAX * 128], bf16,
                                           tag="smT")
                            nc.sync.dma_start(
                                out=smT[:, 0:nm * 128],
                                in_=smT_in[:, m0 * 128:(m0 + nm) * 128])
                            er_ps = erps.tile([128, CHUNK, 4], f32,
                                              space="PSUM", tag="er_ps")
                            for t in range(nt):
                                for (ms, b, ef, el_, _sf, _sl) in \
                                        sched["tiles"][t0 + t]:
                                    lm = ms - m0
                                    nc.tensor.matmul(
                                        er_ps[:, t, :],
                                        lhsT=smT[:, lm * 128:(lm + 1) * 128],
                                        rhs=er_sb[:, b, :],
                                        start=ef, stop=el_)
                            state[ci] = (t0, nt, m0, g, sm, er_ps)

                        def emit_back(ci):
                            (t0, nt, m0, g, sm, er_ps) = state.pop(ci)
                            for g0 in range(0, nt, GROUP):
                                gl = min(GROUP, nt - g0)
                                e4 = xp.tile([128, GROUP, 4], f32, tag="e4")
                                nc.vector.tensor_tensor(
                                    out=e4[:, 0:gl, :],
                                    in0=g[:, g0:g0 + gl, 128:132],
                                    in1=er_ps[:, g0:g0 + gl, :], op=OP.add)
                                lr = xp.tile([128, GROUP, 4], f32, tag="lr")
                                nc.scalar.activation(lr[:, 0:gl, :],
                                                     e4[:, 0:gl, :],
                                                     AF.Prelu, alpha=NEG)
                                ex4 = xp.tile([128, GROUP, 4, 1], f32,
                                              tag="ex4")
                                nc.scalar.activation(ex4[:, 0:gl, :, 0],
                                                     lr[:, 0:gl, :], AF.Exp)
                                m4 = mp.tile([128, GROUP, 132], bf16, tag="m4")
                                nc.scalar.activation(m4[:, 0:gl, 128:132],
                                                     ex4[:, 0:gl, :, 0],
                                                     AF.Copy)
                                nc.vector.tensor_tensor(
                                    out=m4[:, 0:gl, 0:128],
                                    in0=g[:, g0:g0 + gl, 0:128],
                                    in1=ex4[:, 0:gl, :, :]
                                        .to_broadcast([128, gl, 4, 32]),
                                    op=OP.mult)
                                for t in range(gl):
                                    for (ms, b, _ef, _el, sf, sl) in \
                                            sched["tiles"][t0 + g0 + t]:
                                        lm = ms - m0
                                        if sf:
                                            seg_tiles[b] = sgps.tile(
                                                [128, 132], f32, space="PSUM",
                                                tag="seg",
                                                name=f"seg{li}_{passB}_{b}")
                                            if passB:
                                                pa = pap.tile([128, 132], f32,
                                                              tag="pa",
                                                              name=f"pa{li}_{b}")
                                                r = int(ROW_OF[b])
                                                nc.sync.dma_start(
                                                    out=pa[:],
                                                    in_=partialA[
                                                        r * 128:(r + 1) * 128,
                                                        :])
                                                pa_tiles[b] = pa
                                        nc.tensor.matmul(
                                            seg_tiles[b][:],
                                            lhsT=sm[:, lm * 128:(lm + 1) * 128],
                                            rhs=m4[:, t, :],
                                            start=sf, stop=sl)
                                        if sl:
                                            pb = pbp.tile([128, 132], f32,
                                                          tag="pb")
                                            r = int(ROW_OF[b])
                                            if passB:
                                                nc.vector.tensor_tensor(
                                                    out=pb[:],
                                                    in0=seg_tiles[b][:],
                                                    in1=pa_tiles.pop(b)[:],
                                                    op=OP.add)
                                                nc.sync.dma_start(
                                                    out=partial[
                                                        r * 128:(r + 1) * 128,
                                                        :],
                                                    in_=pb[:])
                                                if b in H1_BLOCKS:
                                                    h1rem[0] -= 1
                                                    if h1rem[0] == 0 and \
                                                            not no_cc:
                                                        nc.gpsimd.\
                                                            collective_compute(
                                                            "ReduceScatter",
                                                            mybir.AluOpType.add,
                                                            replica_groups=
                                                            groups_pair,
                                                            ins=[partial[
                                                                0:6144, :]],
                                                            outs=[own_sum[
                                                                0:3072, :]])
                                            else:
                                                nc.scalar.activation(
                                                    pb[:], seg_tiles[b][:],
                                                    AF.Copy)
                                                nc.sync.dma_start(
                                                    out=partialA[
                                                        r * 128:(r + 1) * 128,
                                                        :],
                                                    in_=pb[:])
                                            seg_tiles.pop(b)

                        nchunk = len(sched["chunks"])
                        AHEAD = 2
                        for ci in range(min(AHEAD, nchunk)):
                            emit_front(ci)
                        for ci in range(AHEAD, nchunk):
                            emit_front(ci)
                            emit_back(ci - AHEAD)
                        for ci in range(max(0, nchunk - AHEAD), nchunk):
                            emit_back(ci)

                    edge_pass(schedA, idxA_sb, smA_in, smTA_in, ag_own, False)
                    edge_pass(schedB, idxB_sb, smB_in, smTB_in, table, True)

                if no_cc:
                    nc.sync.dma_start(out=own_sum[0:3072, :],
                                      in_=partial[0:3072, :])
                    nc.sync.dma_start(out=own_sum[3072:OWN, :],
                                      in_=partial[6144:6144 + 3200, :])
                else:
                    nc.gpsimd.collective_compute(
                        "ReduceScatter", mybir.AluOpType.add,
                        replica_groups=groups_pair,
                        ins=[partial[6144:PAIR, :]],
                        outs=[own_sum[3072:OWN, :]])

                # ---- post-processing of own rows (batched) ----
                with tc.tile_pool(name=f"po{li}", bufs=2) as pop, \
                     tc.tile_pool(name=f"pops{li}", bufs=4,
                                  space="PSUM") as tps:
                    osum = pop.tile([128, OWNBLK, 132], f32, name=f"osum{li}",
                                    tag="osum", bufs=1)
                    nc.sync.dma_start(
                        out=osum[:],
                        in_=own_sum[:, :].rearrange("(t p) c -> p t c", p=128))
                    den = pop.tile([128, OWNBLK, 4], f32, tag="den", bufs=1)
                    nc.vector.tensor_scalar_max(den[:], osum[:, :, 128:132],
                                                EPS)
                    rec = pop.tile([128, OWNBLK, 4, 1], f32, tag="rec", bufs=1)
                    nc.vector.reciprocal(rec[:, :, :, 0], den[:])
                    if not last:
                        o2 = pop.tile([128, OWNBLK, 128], f32, tag="o2",
                                      bufs=1)
                        nc.vector.tensor_tensor(
                            out=o2[:].rearrange("p t (h d) -> p t h d", h=4),
                            in0=osum[:, :, 0:128]
                                .rearrange("p t (h d) -> p t h d", h=4),
                            in1=rec[:].to_broadcast([128, OWNBLK, 4, 32]),
                            op=OP.mult)
                        nc.vector.tensor_tensor(
                            out=o2[:], in0=o2[:],
                            in1=brep_sb[li][:]
                                .rearrange("p (t c) -> p t c", t=1)
                                .to_broadcast([128, OWNBLK, 128]),
                            op=OP.add)
                        # ELU via scalar engine: exn = exp(-relu(-x)) =
                        # exp(min(x,0)); o2 = relu(x) + exn - 1
                        exn = pop.tile([128, OWNBLK, 128], f32, tag="exn",
                                       bufs=1)
                        nc.scalar.activation(exn[:], o2[:], AF.Relu,
                                             scale=-1.0)
                        nc.scalar.activation(exn[:], exn[:], AF.Exp,
                                             scale=-1.0)
                        nc.scalar.activation(o2[:], o2[:], AF.Relu)
                        nc.vector.tensor_tensor(out=o2[:], in0=o2[:],
                                                in1=exn[:], op=OP.add)
                        nc.scalar.activation(o2[:], o2[:], AF.Copy, bias=-1.0)
                        dst_hT = hT2 if li % 2 == 0 else hT
                        for b in range(OWNBLK):
                            tp = tps.tile([128, 128], f32, space="PSUM")
                            nc.tensor.matmul(tp[:], lhsT=o2[:, b, :],
                                             rhs=ident_sb[:], start=True,
                                             stop=True)
                            nc.scalar.activation(
                                dst_hT[:, b * 128:(b + 1) * 128],
                                tp[:], AF.Copy)
                    else:
                        r4 = pop.tile([128, OWNBLK, 4, 32], f32, tag="r4",
                                      bufs=1)
                        nc.vector.tensor_tensor(
                            out=r4[:],
                            in0=osum[:, :, 0:128]
                                .rearrange("p t (h d) -> p t h d", h=4),
                            in1=rec[:].to_broadcast([128, OWNBLK, 4, 32]),
                            op=OP.mult)
                        r1 = pop.tile([128, OWNBLK, 32], f32, tag="r1", bufs=1)
                        nc.vector.tensor_tensor(out=r1[:], in0=r4[:, :, 0, :],
                                                in1=r4[:, :, 1, :], op=OP.add)
                        r2 = pop.tile([128, OWNBLK, 32], f32, tag="r2", bufs=1)
                        nc.vector.tensor_tensor(out=r2[:], in0=r4[:, :, 2, :],
                                                in1=r4[:, :, 3, :], op=OP.add)
                        nc.vector.tensor_tensor(out=r1[:], in0=r1[:],
                                                in1=r2[:], op=OP.add)
                        nc.vector.tensor_scalar_mul(r1[:], r1[:], 0.25)
                        nc.vector.tensor_tensor(
                            out=r1[:], in0=r1[:],
                            in1=brep_sb[li][:]
                                .rearrange("p (t c) -> p t c", t=1)
                                .to_broadcast([128, OWNBLK, 32]),
                            op=OP.add)
                        nc.sync.dma_start(
                            out=y_out[0:6144, :]
                                .rearrange("(t p) c -> p t c", p=128),
                            in_=r1[:, 0:48, :])
                        nc.sync.dma_start(
                            out=y_out[6144:NPC, :],
                            in_=r1[0:NPC - 6144, 48, :])

                if not last:
                    proj_phase(li + 1, hT2 if li % 2 == 0 else hT)
    nc.finalize()
    return nc


def _make_consts(W0, al0, ar0, b0, W1, al1, ar1, b1, W2, al2, ar2, b2):
    consts = {}
    for li, (W, al, ar, b) in enumerate(
            [(W0, al0, ar0, b0), (W1, al1, ar1, b1), (W2, al2, ar2, b2)]):
        consts[f"Waug{li}"] = _augment(np.asarray(W, np.float32),
                                       np.asarray(al, np.float32),
                                       np.asarray(ar, np.float32))
        b = np.asarray(b, np.float32)
        if li < 2:
            consts[f"brep{li}"] = np.tile(b.reshape(1, 128), (128, 1))
        else:
            consts[f"brep{li}"] = np.tile(b.reshape(H, D).mean(0).reshape(1, D),
                                          (128, 1))
    dummy = np.zeros((1, TCOLS), ml_dtypes.bfloat16)
    dummy[0, 128:132] = ml_dtypes.bfloat16(-1e30)
    consts["dummyrow"] = dummy
    return consts


def _in_maps(x):
    cores = _cache["pre"][0]
    x = np.asarray(x, dtype=np.float32)
    in_maps = []
    for c in range(NCORE):
        lo, hi = _own_rows(c)
        xT = np.zeros((128, OWN), np.float32)
        xT[:, 0:NPC] = x[lo:hi].T
        cc = cores[c]
        in_maps.append(dict(xT=xT, idxA=cc["idxA"], idxB=cc["idxB"],
                            smA=np.asarray(cc["smA"]),
                            smTA=np.asarray(cc["smTA"]),
                            smB=np.asarray(cc["smB"]),
                            smTB=np.asarray(cc["smTB"])))
    return in_maps


def kernel(x, src, dst, W0, al0, ar0, b0, W1, al1, ar1, b1, W2, al2, ar2, b2):
    from concourse.bass_utils import run_bass_kernel_spmd

    key = (hash(np.asarray(src).tobytes()) ^ hash(np.asarray(dst).tobytes()))
    if "pre" not in _cache or _cache.get("prekey") != key:
        _cache["pre"] = _preprocess(src, dst)
        _cache["prekey"] = key
    cores, schedA, schedB = _cache["pre"]

    consts = _make_consts(W0, al0, ar0, b0, W1, al1, ar1, b1, W2, al2, ar2, b2)

    ck = key ^ hash(consts["Waug0"].tobytes())
    if "nc" not in _cache or _cache.get("nckey") != ck:
        _cache["nc"] = _build(schedA, schedB, consts)
        _cache["nckey"] = ck
    nc = _cache["nc"]

    in_maps = _in_maps(x)
    r = run_bass_kernel_spmd(nc, in_maps, list(range(NCORE)))
    y = np.zeros((N, D), np.float32)
    for c in range(NCORE):
        lo, hi = _own_rows(c)
        y[lo:hi] = r.results[c]["y"]
    return y


# revision 18
# speedup vs baseline: 1.0330x; 1.0330x over previous
"""3-layer GAT on 8 Trainium2 NeuronCores (Bass/Tile).

Sharding: 2D graph partition. Pair q = cores {2q, 2q+1} aggregates the dst
nodes of strips [q*6250,(q+1)*6250) and [25000+q*6250, 25000+(q+1)*6250);
even cores take edges with src < 25000, odd cores the rest. Node ownership:
core 2k owns rows [k*6250,(k+1)*6250), core 2k+1 owns [25000+k*6250, ...).

Per layer: each core projects its own rows (feat|el|er via an augmented
weight matrix) into a local gather table, then runs the edge phase in two
passes: pass A covers edges whose source is one of the core's own rows and
gathers from the local table while the quad AllGather of the full src-half
table is still in flight; pass B covers the remaining edges and gathers
from the AllGathered table. Edges are packed into 128-wide tiles grouped
by pairs of 128-dst blocks (a tile may straddle the two blocks; the
host-precomputed one-hot masks select membership). Per-edge er comes from
a transposed one-hot matmul against SBUF-resident per-block er rows (no
second gather). Messages are accumulated per dst block by one-hot-mask
matmuls into PSUM; pass B adds pass A's partial sums back in. Partial sums
are pairwise ReduceScattered in two halves (the first fires mid-pass-B),
then a batched divide/bias/ELU (head-mean on the last layer) produces the
output rows, fused with the next layer's projection.
"""

import numpy as np
import ml_dtypes

N = 50000
E = 800000
F = 128                  # input feats and hidden width (4 heads x 32)
H = 4
D = 32
NEG = 0.2
NCORE = 8
NPC = 6250               # nodes owned per core
OWN = 6272               # 49*128, padded own rows
OWNBLK = 49
PAIR = 12544             # 98*128 dst slots per pair
NBLK = 98
NGRP = 49                # pair-groups of 2 blocks
HALF = 25088             # 4*OWN rows per src-half table
TROWS = 25216            # HALF + 128 (dummy row at HALF)
DUMMY = HALF
TCOLS = 256              # bf16 cols: feat(128) | el(4) | pad
CHUNK = 32               # max tiles per dma_gather call
GROUP = 8                # tiles per vector-op batch
EPS = 1e-30

# pair-group processing order: finish half-1 blocks ({0..23} u {49..72})
# first so the first ReduceScatter can fire mid-pass.
GORDER = list(range(0, 12)) + list(range(24, 37)) + \
         list(range(12, 24)) + list(range(37, 49))
H1_BLOCKS = set(range(0, 24)) | set(range(49, 73))

# block -> partial-row-block permutation: [A1(24) B1(24) A2(25) B2(25)]
ROW_OF = np.empty(NBLK, np.int64)
ROW_OF[0:24] = np.arange(24)
ROW_OF[49:73] = 24 + np.arange(24)
ROW_OF[24:49] = 48 + np.arange(25)
ROW_OF[73:98] = 73 + np.arange(25)

_cache = {}


def _schedule(cnt):
    """Core-uniform tile/mask schedule for one pass.

    cnt: [NCORE, NBLK] per-core per-block edge counts.
    Returns dict with T, nmask, chunks, tiles (per tile: list of
    (mslot, block, er_first, er_last, sc_first, sc_last)).
    """
    n0 = cnt[:, 0::2]                      # [NCORE, NGRP]
    n1 = cnt[:, 1::2]
    TP = np.maximum(1, np.ceil((n0 + n1).max(axis=0) / 128).astype(np.int64))

    base_tile = {}
    acc = 0
    for g in GORDER:
        base_tile[g] = acc
        acc += int(TP[g])
    T = acc

    tiles = []            # per tile: list of [mslot, block]
    tile_group = []
    # which (tile-in-group, block-parity) pairs are needed on any core; ensure
    # every block gets at least one occurrence (tile 0 fallback)
    need = {}
    for g in GORDER:
        for i in range(int(TP[g])):
            need[(g, i, 0)] = bool((n0[:, g] > 128 * i).any())
            need[(g, i, 1)] = bool(
                ((n0[:, g] < 128 * (i + 1)) &
                 (n0[:, g] + n1[:, g] > 128 * i)).any())
        if not any(need[(g, i, 0)] for i in range(int(TP[g]))):
            need[(g, 0, 0)] = True
        if not any(need[(g, i, 1)] for i in range(int(TP[g]))):
            need[(g, 0, 1)] = True
    mslot = 0
    for g in GORDER:
        for i in range(int(TP[g])):
            ml = []
            if need[(g, i, 0)]:
                ml.append([mslot, 2 * g])
                mslot += 1
            if need[(g, i, 1)]:
                ml.append([mslot, 2 * g + 1])
                mslot += 1
            assert ml
            tiles.append(ml)
            tile_group.append(g)
    nmask = mslot

    # per-block first/last occurrence
    occ = {}
    for ti, ml in enumerate(tiles):
        for m in ml:
            occ.setdefault(m[1], []).append((ti, m[0]))
    first = {b: o[0] for b, o in occ.items()}
    last = {b: o[-1] for b, o in occ.items()}
    sched_tiles = []
    for ti, ml in enumerate(tiles):
        entry = []
        for k, (ms, b) in enumerate(ml):
            entry.append((ms, b,
                          k == 0, k == len(ml) - 1,
                          first[b] == (ti, ms), last[b] == (ti, ms)))
        sched_tiles.append(entry)

    # chunks aligned to pair-group boundaries, up to CHUNK tiles
    chunks = []
    t0 = 0
    ti = 0
    for g in GORDER:
        ti += int(TP[g])
        nxt = None
        gi = GORDER.index(g)
        if gi + 1 < len(GORDER):
            nxt = int(TP[GORDER[gi + 1]])
        if nxt is None or ti - t0 + nxt > CHUNK:
            m0 = min(m[0] for m in sched_tiles[t0]) if sched_tiles[t0] else 0
            mend = max(m[0] for m in sched_tiles[ti - 1]) + 1
            chunks.append((t0, ti - t0, m0, mend - m0))
            t0 = ti
    assert t0 == T
    return dict(T=T, nmask=nmask, chunks=chunks, tiles=sched_tiles,
                tile_group=tile_group, base_tile=base_tile, TP=TP)


def _wrap16(a):
    # value i of each 128-group at [i%16, i//16], replicated per 16 rows
    t = a.reshape(-1, 128)                     # [T, 128]
    w = t.reshape(t.shape[0], 8, 16)           # [T, 8, 16]
    w = w.transpose(2, 0, 1).reshape(16, -1)   # [16, T*8]
    return np.tile(w, (8, 1)).astype(np.int16)  # [128, T*8]


def _core_pass_arrays(sched, rloc_e, rows_e, pad_row):
    """Build idx + mask streams for one (core, pass).

    rloc_e: pair-local dst row per edge; rows_e: gather-table row per edge.
    """
    T, nmask = sched["T"], sched["nmask"]
    base_tile = sched["base_tile"]
    # group rank of each edge
    grank_of = np.empty(NGRP, np.int64)
    for r, g in enumerate(GORDER):
        grank_of[g] = r
    pg = rloc_e // 256
    gr = grank_of[pg]
    order = np.lexsort((rloc_e, gr))
    rloc_s = rloc_e[order]
    rows_s = rows_e[order]
    gr_s = gr[order]
    # position within group
    starts = np.searchsorted(gr_s, np.arange(len(GORDER)))
    pos_in_group = np.arange(len(gr_s)) - starts[gr_s]
    base128 = np.array([base_tile[GORDER[r]] * 128
                        for r in range(len(GORDER))], np.int64)
    s_glob = base128[gr_s] + pos_in_group

    idx = np.full(T * 128, pad_row, np.int64)
    idx[s_glob] = rows_s

    # mask slot lookup per (tile, block-parity)
    mslot_of = np.full((T, 2), -1, np.int64)
    for ti, ml in enumerate(sched["tiles"]):
        g = sched["tile_group"][ti]
        for (ms, b, *_fl) in ml:
            mslot_of[ti, b - 2 * g] = ms
    ti_e = s_glob // 128
    e_e = s_glob % 128
    b_e = rloc_s // 128
    s128_e = rloc_s % 128
    par = b_e - 2 * np.array(sched["tile_group"])[ti_e]
    ms_e = mslot_of[ti_e, par]
    assert (ms_e >= 0).all()

    smatw = np.zeros((128, nmask * 128), ml_dtypes.bfloat16)
    smatw[e_e, ms_e * 128 + s128_e] = 1
    smatTw = np.zeros((128, nmask * 128), ml_dtypes.bfloat16)
    smatTw[s128_e, ms_e * 128 + e_e] = 1
    return _wrap16(idx), smatw, smatTw


def _preprocess(src, dst):
    src = np.asarray(src).astype(np.int64)
    dst = np.asarray(dst).astype(np.int64)
    q = np.where(dst < 25000, dst // NPC, (dst - 25000) // NPC)
    s = (src >= 25000).astype(np.int64)
    core_of = 2 * q + s
    rloc = np.where(dst < 25000, dst - q * NPC, OWN + (dst - 25000 - q * NPC))
    ks = np.where(src < 25000, src // NPC, (src - 25000) // NPC)
    tloc = np.where(src < 25000, OWN * ks + src - ks * NPC,
                    OWN * ks + (src - 25000) - ks * NPC)
    own = ks == q
    ownrow = np.where(src < 25000, src - ks * NPC, src - 25000 - ks * NPC)
    blk = rloc // 128

    cntA = np.zeros((NCORE, NBLK), np.int64)
    cntB = np.zeros((NCORE, NBLK), np.int64)
    for c in range(NCORE):
        m = core_of == c
        cntA[c] = np.bincount(blk[m & own], minlength=NBLK)
        cntB[c] = np.bincount(blk[m & ~own], minlength=NBLK)
    schedA = _schedule(cntA)
    schedB = _schedule(cntB)

    cores = []
    for c in range(NCORE):
        m = core_of == c
        mA = m & own
        mB = m & ~own
        idxA, smA, smTA = _core_pass_arrays(schedA, rloc[mA], ownrow[mA], OWN)
        idxB, smB, smTB = _core_pass_arrays(schedB, rloc[mB], tloc[mB], DUMMY)
        cores.append(dict(idxA=idxA, smA=smA, smTA=smTA,
                          idxB=idxB, smB=smB, smTB=smTB))
    return cores, schedA, schedB


def _own_rows(c):
    k = c // 2
    if c % 2 == 0:
        return k * NPC, (k + 1) * NPC
    return 25000 + k * NPC, 25000 + (k + 1) * NPC


def _augment(W, al, ar):
    dout = W.shape[1] // H
    Wal = np.stack([W[:, h * dout:(h + 1) * dout] @ al[h] for h in range(H)], 1)
    War = np.stack([W[:, h * dout:(h + 1) * dout] @ ar[h] for h in range(H)], 1)
    return np.concatenate([W, Wal, War], 1).astype(np.float32)  # [128, 136]


def _build(schedA, schedB, consts, no_cc=False):
    import concourse.bass as bass
    import concourse.bacc as bacc
    import concourse.tile as tile
    from concourse import mybir
    from concourse.library_config import mlp

    f32 = mybir.dt.float32
    bf16 = mybir.dt.bfloat16
    i16 = mybir.dt.int16
    AF = mybir.ActivationFunctionType
    OP = mybir.AluOpType

    TA, TB = schedA["T"], schedB["T"]
    NMA, NMB = schedA["nmask"], schedB["nmask"]
    NM_MAX = max(max(nm for (_, _, _, nm) in schedA["chunks"]),
                 max(nm for (_, _, _, nm) in schedB["chunks"]))

    nc = bacc.Bacc(num_devices=NCORE)
    xT_in = nc.declare_dram_parameter("xT", [128, OWN], bf16, isOutput=False)
    idxA_in = nc.declare_dram_parameter("idxA", [128, TA * 8], i16,
                                        isOutput=False)
    idxB_in = nc.declare_dram_parameter("idxB", [128, TB * 8], i16,
                                        isOutput=False)
    smA_in = nc.declare_dram_parameter("smA", [128, NMA * 128], bf16,
                                       isOutput=False)
    smTA_in = nc.declare_dram_parameter("smTA", [128, NMA * 128], bf16,
                                        isOutput=False)
    smB_in = nc.declare_dram_parameter("smB", [128, NMB * 128], bf16,
                                       isOutput=False)
    smTB_in = nc.declare_dram_parameter("smTB", [128, NMB * 128], bf16,
                                        isOutput=False)
    y_out = nc.declare_dram_parameter("y", [NPC, D], f32, isOutput=True)

    with tile.TileContext(nc) as tc:
        with tc.tile_pool(name="persist", bufs=1) as pp, \
             tc.tile_pool(name="dram", bufs=1, space="DRAM") as dp:
            nc.gpsimd.load_library(mlp)

            # ---- persistent SBUF state ----
            idxA_sb = pp.tile([128, TA * 8], i16)
            nc.sync.dma_start(out=idxA_sb[:], in_=idxA_in[:, :])
            idxB_sb = pp.tile([128, TB * 8], i16)
            nc.sync.dma_start(out=idxB_sb[:], in_=idxB_in[:, :])
            hT = pp.tile([128, OWN], bf16)
            nc.sync.dma_start(out=hT[:], in_=xT_in[:, :])
            hT2 = pp.tile([128, OWN], bf16)

            ident_h = nc.inline_tensor(np.eye(128, dtype=np.float32),
                                       name="ident")
            ident_sb = pp.tile([128, 128], f32)
            nc.sync.dma_start(out=ident_sb[:], in_=ident_h[:, :])

            waug_sb = []
            brep_sb = []
            for li in range(3):
                wh = nc.inline_tensor(consts[f"Waug{li}"], name=f"waug{li}")
                wt = pp.tile([128, 136], bf16, name=f"waug_sb{li}")
                nc.sync.dma_start(out=wt[:], in_=wh[:, :])
                waug_sb.append(wt)
                bh = nc.inline_tensor(consts[f"brep{li}"], name=f"brep{li}")
                bt = pp.tile([128, consts[f"brep{li}"].shape[1]], f32,
                             name=f"brep_sb{li}")
                nc.sync.dma_start(out=bt[:], in_=bh[:, :])
                brep_sb.append(bt)

            dummy_h = nc.inline_tensor(consts["dummyrow"], name="dummyrow")

            # ---- DRAM scratch ----
            table = dp.tile([TROWS, TCOLS], bf16)
            er_tab = dp.tile([PAIR, 4], bf16)
            ag_own = dp.tile([OWN + 128, TCOLS], bf16)
            ag_er = dp.tile([OWN, 4], bf16)
            partialA = dp.tile([PAIR, 132], f32)
            partial = dp.tile([PAIR, 132], f32)
            own_sum = dp.tile([OWN, 132], f32)

            nc.sync.dma_start(out=table[DUMMY:DUMMY + 1, :], in_=dummy_h[:, :])
            nc.sync.dma_start(out=ag_own[OWN:OWN + 1, :], in_=dummy_h[:, :])

            groups_pair = [[2 * k, 2 * k + 1] for k in range(4)]
            groups_quad = [[0, 2, 4, 6], [1, 3, 5, 7]]

            def proj_phase(li, src_hT):
                with tc.tile_pool(name=f"prj{li}", bufs=3) as sp, \
                     tc.tile_pool(name=f"prjps{li}", bufs=3,
                                  space="PSUM") as ps:
                    # er-only mini-projection first so the small er AllGather
                    # launches early (pass A consumes it)
                    errow = sp.tile([128, OWNBLK, 4], bf16, name=f"errow{li}",
                                    tag="errow", bufs=1)
                    for t in range(OWNBLK):
                        pje = ps.tile([128, 4], f32, space="PSUM", tag="pje")
                        nc.tensor.matmul(pje[:],
                                         lhsT=src_hT[:, t * 128:(t + 1) * 128],
                                         rhs=waug_sb[li][:, 132:136],
                                         start=True, stop=True)
                        nc.scalar.activation(errow[:, t, :], pje[:], AF.Copy)
                    nc.sync.dma_start(
                        out=ag_er[:, :].rearrange("(t p) c -> p t c", p=128),
                        in_=errow[:])
                    if no_cc:
                        for rep in range(2):
                            nc.sync.dma_start(
                                out=er_tab[rep * OWN:(rep + 1) * OWN, :],
                                in_=ag_er[:, :])
                    else:
                        nc.gpsimd.collective_compute(
                            "AllGather", mybir.AluOpType.bypass,
                            replica_groups=groups_pair,
                            ins=[ag_er[:, :]], outs=[er_tab[:, :]])

                    tabrow = sp.tile([128, OWNBLK, TCOLS], bf16,
                                     name=f"tabrow{li}", tag="tabrow", bufs=1)
                    for t in range(OWNBLK):
                        pj = ps.tile([128, 132], f32, space="PSUM", tag="pj")
                        nc.tensor.matmul(pj[:],
                                         lhsT=src_hT[:, t * 128:(t + 1) * 128],
                                         rhs=waug_sb[li][:, 0:132], start=True,
                                         stop=True)
                        nc.scalar.activation(tabrow[:, t, 0:132], pj[:],
                                             AF.Copy)
                    nc.sync.dma_start(
                        out=ag_own[0:OWN, :]
                            .rearrange("(t p) c -> p t c", p=128),
                        in_=tabrow[:])
                if no_cc:
                    for rep in range(4):
                        nc.sync.dma_start(
                            out=table[rep * OWN:(rep + 1) * OWN, :],
                            in_=ag_own[0:OWN, :])
                else:
                    nc.gpsimd.collective_compute(
                        "AllGather", mybir.AluOpType.bypass,
                        replica_groups=groups_quad,
                        ins=[ag_own[0:OWN, :]], outs=[table[0:HALF, :]])

            proj_phase(0, hT)

            for li in range(3):
                last = li == 2

                with tc.tile_pool(name=f"gt{li}", bufs=4) as gp, \
                     tc.tile_pool(name=f"mk{li}", bufs=3) as mkp, \
                     tc.tile_pool(name=f"ms{li}", bufs=4) as mp, \
                     tc.tile_pool(name=f"ex{li}", bufs=4) as xp, \
                     tc.tile_pool(name=f"pb{li}", bufs=4) as pbp, \
                     tc.tile_pool(name=f"pa{li}", bufs=4) as pap, \
                     tc.tile_pool(name=f"sg{li}", bufs=4,
                                  space="PSUM") as sgps, \
                     tc.tile_pool(name=f"er{li}", bufs=3,
                                  space="PSUM") as erps:
                    er_sb = mp.tile([128, NBLK, 4], bf16, tag="er_sb", bufs=1,
                                    name=f"er_sb{li}")
                    nc.sync.dma_start(
                        out=er_sb[:],
                        in_=er_tab[:, :].rearrange("(t p) c -> p t c", p=128))

                    def edge_pass(sched, idx_sb, sm_in, smT_in, tab, passB):
                        seg_tiles = {}
                        pa_tiles = {}
                        h1rem = [len(H1_BLOCKS)]
                        state = {}

                        def emit_front(ci):
                            # gather + mask streams + er matmuls for chunk ci
                            (t0, nt, m0, nm) = sched["chunks"][ci]
                            g = gp.tile([128, CHUNK, TCOLS], bf16, tag="g")
                            nc.gpsimd.dma_gather(
                                out_ap=g[:, 0:nt, :], in_ap=tab[:, :],
                                idxs_ap=idx_sb[:, t0 * 8:(t0 + nt) * 8],
                                num_idxs=nt * 128, num_idxs_reg=nt * 128,
                                elem_size=TCOLS, single_packet=False)
                            sm = mkp.tile([128, NM_MAX * 128], bf16, tag="sm")
                            nc.sync.dma_start(
                                out=sm[:, 0:nm * 128],
                                in_=sm_in[:, m0 * 128:(m0 + nm) * 128])
                            smT = mkp.tile([128, NM_MAX * 128], bf16,
                                           tag="smT")
                            nc.sync.dma_start(
                                out=smT[:, 0:nm * 128],
                                in_=smT_in[:, m0 * 128:(m0 + nm) * 128])
                            er_ps = erps.tile([128, CHUNK, 4], f32,
                                              space="PSUM", tag="er_ps")
                            for t in range(nt):
                                for (ms, b, ef, el_, _sf, _sl) in \
                                        sched["tiles"][t0 + t]:
                                    lm = ms - m0
                                    nc.tensor.matmul(
                                        er_ps[:, t, :],
                                        lhsT=smT[:, lm * 128:(lm + 1) * 128],
                                        rhs=er_sb[:, b, :],
                                        start=ef, stop=el_)
                            state[ci] = (t0, nt, m0, g, sm, er_ps)

                        def emit_back(ci):
                            (t0, nt, m0, g, sm, er_ps) = state.pop(ci)
                            for g0 in range(0, nt, GROUP):
                                gl = min(GROUP, nt - g0)
                                e4 = xp.tile([128, GROUP, 4], f32, tag="e4")
                                nc.vector.tensor_tensor(
                                    out=e4[:, 0:gl, :],
                                    in0=g[:, g0:g0 + gl, 128:132],
                                    in1=er_ps[:, g0:g0 + gl, :], op=OP.add)
                                lr = xp.tile([128, GROUP, 4], f32, tag="lr")
                                nc.scalar.activation(lr[:, 0:gl, :],
                                                     e4[:, 0:gl, :],
                                                     AF.Prelu, alpha=NEG)
                                ex4 = xp.tile([128, GROUP, 4, 1], f32,
                                              tag="ex4")
                                nc.scalar.activation(ex4[:, 0:gl, :, 0],
                                                     lr[:, 0:gl, :], AF.Exp)
                                m4 = mp.tile([128, GROUP, 132], bf16, tag="m4")
                                nc.scalar.activation(m4[:, 0:gl, 128:132],
                                                     ex4[:, 0:gl, :, 0],
                                                     AF.Copy)
                                nc.vector.tensor_tensor(
                                    out=m4[:, 0:gl, 0:128],
                                    in0=g[:, g0:g0 + gl, 0:128],
                                    in1=ex4[:, 0:gl, :, :]
                                        .to_broadcast([128, gl, 4, 32]),
                                    op=OP.mult)
                                for t in range(gl):
                                    for (ms, b, _ef, _el, sf, sl) in \
                                            sched["tiles"][t0 + g0 + t]:
                                        lm = ms - m0
                                        if sf:
                                            seg_tiles[b] = sgps.tile(
                                                [128, 132], f32, space="PSUM",
                                                tag="seg",
                                                name=f"seg{li}_{passB}_{b}")
                                            if passB:
                                                pa = pap.tile([128, 132], f32,
                                                              tag="pa",
                                                              name=f"pa{li}_{b}")
                                                r = int(ROW_OF[b])
                                                nc.sync.dma_start(
                                                    out=pa[:],
                                                    in_=partialA[
                                                        r * 128:(r + 1) * 128,
                                                        :])
                                                pa_tiles[b] = pa
                                        nc.tensor.matmul(
                                            seg_tiles[b][:],
                                            lhsT=sm[:, lm * 128:(lm + 1) * 128],
                                            rhs=m4[:, t, :],
                                            start=sf, stop=sl)
                                        if sl:
                                            pb = pbp.tile([128, 132], f32,
                                                          tag="pb")
                                            r = int(ROW_OF[b])
                                            if passB:
                                                nc.vector.tensor_tensor(
                                                    out=pb[:],
                                                    in0=seg_tiles[b][:],
                                                    in1=pa_tiles.pop(b)[:],
                                                    op=OP.add)
                                                nc.sync.dma_start(
                                                    out=partial[
                                                        r * 128:(r + 1) * 128,
                                                        :],
                                                    in_=pb[:])
                                                if b in H1_BLOCKS:
                                                    h1rem[0] -= 1
                                                    if h1rem[0] == 0 and \
                                                            not no_cc:
                                                        nc.gpsimd.\
                                                            collective_compute(
                                                            "ReduceScatter",
                                                            mybir.AluOpType.add,
                                                            replica_groups=
                                                            groups_pair,
                                                            ins=[partial[
                                                                0:6144, :]],
                                                            outs=[own_sum[
                                                                0:3072, :]])
                                            else:
                                                nc.scalar.activation(
                                                    pb[:], seg_tiles[b][:],
                                                    AF.Copy)
                                                nc.sync.dma_start(
                                                    out=partialA[
                                                        r * 128:(r + 1) * 128,
                                                        :],
                                                    in_=pb[:])
                                            seg_tiles.pop(b)

                        nchunk = len(sched["chunks"])
                        AHEAD = 2
                        for ci in range(min(AHEAD, nchunk)):
                            emit_front(ci)
                        for ci in range(AHEAD, nchunk):
                            emit_front(ci)
                            emit_back(ci - AHEAD)
                        for ci in range(max(0, nchunk - AHEAD), nchunk):
                            emit_back(ci)

                    edge_pass(schedA, idxA_sb, smA_in, smTA_in, ag_own, False)
                    edge_pass(schedB, idxB_sb, smB_in, smTB_in, table, True)

                if no_cc:
                    nc.sync.dma_start(out=own_sum[0:3072, :],
                                      in_=partial[0:3072, :])
                    nc.sync.dma_start(out=own_sum[3072:OWN, :],
                                      in_=partial[6144:6144 + 3200, :])
                else:
                    nc.gpsimd.collective_compute(
                        "ReduceScatter", mybir.AluOpType.add,
                        replica_groups=groups_pair,
                        ins=[partial[6144:PAIR, :]],
                        outs=[own_sum[3072:OWN, :]])

                # ---- post-processing of own rows (batched) ----
                with tc.tile_pool(name=f"po{li}", bufs=2) as pop, \
                     tc.tile_pool(name=f"pops{li}", bufs=4,
                                  space="PSUM") as tps:
                    osum = pop.tile([128, OWNBLK, 132], f32, name=f"osum{li}",
                                    tag="osum", bufs=1)
                    nc.sync.dma_start(
                        out=osum[:],
                        in_=own_sum[:, :].rearrange("(t p) c -> p t c", p=128))
                    den = pop.tile([128, OWNBLK, 4], f32, tag="den", bufs=1)
                    nc.vector.tensor_scalar_max(den[:], osum[:, :, 128:132],
                                                EPS)
                    rec = pop.tile([128, OWNBLK, 4, 1], f32, tag="rec", bufs=1)
                    nc.vector.reciprocal(rec[:, :, :, 0], den[:])
                    if not last:
                        o2 = pop.tile([128, OWNBLK, 128], f32, tag="o2",
                                      bufs=1)
                        nc.vector.tensor_tensor(
                            out=o2[:].rearrange("p t (h d) -> p t h d", h=4),
                            in0=osum[:, :, 0:128]
                                .rearrange("p t (h d) -> p t h d", h=4),
                            in1=rec[:].to_broadcast([128, OWNBLK, 4, 32]),
                            op=OP.mult)
                        nc.vector.tensor_tensor(
                            out=o2[:], in0=o2[:],
                            in1=brep_sb[li][:]
                                .rearrange("p (t c) -> p t c", t=1)
                                .to_broadcast([128, OWNBLK, 128]),
                            op=OP.add)
                        # ELU via scalar engine: exn = exp(-relu(-x)) =
                        # exp(min(x,0)); o2 = relu(x) + exn - 1
                        exn = pop.tile([128, OWNBLK, 128], f32, tag="exn",
                                       bufs=1)
                        nc.scalar.activation(exn[:], o2[:], AF.Relu,
                                             scale=-1.0)
                        nc.scalar.activation(exn[:], exn[:], AF.Exp,
                                             scale=-1.0)
                        nc.scalar.activation(o2[:], o2[:], AF.Relu)
                        nc.vector.tensor_tensor(out=o2[:], in0=o2[:],
                                                in1=exn[:], op=OP.add)
                        nc.scalar.activation(o2[:], o2[:], AF.Copy, bias=-1.0)
                        dst_hT = hT2 if li % 2 == 0 else hT
                        for b in range(OWNBLK):
                            tp = tps.tile([128, 128], f32, space="PSUM")
                            nc.tensor.matmul(tp[:], lhsT=o2[:, b, :],
                                             rhs=ident_sb[:], start=True,
                                             stop=True)
                            nc.scalar.activation(
                                dst_hT[:, b * 128:(b + 1) * 128],
                                tp[:], AF.Copy)
                    else:
                        r4 = pop.tile([128, OWNBLK, 4, 32], f32, tag="r4",
                                      bufs=1)
                        nc.vector.tensor_tensor(
                            out=r4[:],
                            in0=osum[:, :, 0:128]
                                .rearrange("p t (h d) -> p t h d", h=4),
                            in1=rec[:].to_broadcast([128, OWNBLK, 4, 32]),
                            op=OP.mult)
                        r1 = pop.tile([128, OWNBLK, 32], f32, tag="r1", bufs=1)
                        nc.vector.tensor_tensor(out=r1[:], in0=r4[:, :, 0, :],
                                                in1=r4[:, :, 1, :], op=OP.add)
                        r2 = pop.tile([128, OWNBLK, 32], f32, tag="r2", bufs=1)
                        nc.vector.tensor_tensor(out=r2[:], in0=r4[:, :, 2, :],
                                                in1=r4[:, :, 3, :], op=OP.add)
                        nc.vector.tensor_tensor(out=r1[:], in0=r1[:],
                                                in1=r2[:], op=OP.add)
                        nc.vector.tensor_scalar_mul(r1[:], r1[:], 0.25)
                        nc.vector.tensor_tensor(
                            out=r1[:], in0=r1[:],
                            in1=brep_sb[li][:]
                                .rearrange("p (t c) -> p t c", t=1)
                                .to_broadcast([128, OWNBLK, 32]),
                            op=OP.add)
                        nc.sync.dma_start(
                            out=y_out[0:6144, :]
                                .rearrange("(t p) c -> p t c", p=128),
                            in_=r1[:, 0:48, :])
                        nc.sync.dma_start(
                            out=y_out[6144:NPC, :],
                            in_=r1[0:NPC - 6144, 48, :])

                if not last:
                    proj_phase(li + 1, hT2 if li % 2 == 0 else hT)
    nc.finalize()
    return nc


def _make_consts(W0, al0, ar0, b0, W1, al1, ar1, b1, W2, al2, ar2, b2):
    consts = {}
    for li, (W, al, ar, b) in enumerate(
            [(W0, al0, ar0, b0), (W1, al1, ar1, b1), (W2, al2, ar2, b2)]):
        consts[f"Waug{li}"] = _augment(np.asarray(W, np.float32),
                                       np.asarray(al, np.float32),
                                       np.asarray(ar, np.float32)).astype(
                                           ml_dtypes.bfloat16)
        b = np.asarray(b, np.float32)
        if li < 2:
            consts[f"brep{li}"] = np.tile(b.reshape(1, 128), (128, 1))
        else:
            consts[f"brep{li}"] = np.tile(b.reshape(H, D).mean(0).reshape(1, D),
                                          (128, 1))
    dummy = np.zeros((1, TCOLS), ml_dtypes.bfloat16)
    dummy[0, 128:132] = ml_dtypes.bfloat16(-1e30)
    consts["dummyrow"] = dummy
    return consts


def _in_maps(x):
    cores = _cache["pre"][0]
    x = np.asarray(x, dtype=np.float32)
    in_maps = []
    for c in range(NCORE):
        lo, hi = _own_rows(c)
        xT = np.zeros((128, OWN), ml_dtypes.bfloat16)
        xT[:, 0:NPC] = x[lo:hi].T.astype(ml_dtypes.bfloat16)
        cc = cores[c]
        in_maps.append(dict(xT=xT, idxA=cc["idxA"], idxB=cc["idxB"],
                            smA=np.asarray(cc["smA"]),
                            smTA=np.asarray(cc["smTA"]),
                            smB=np.asarray(cc["smB"]),
                            smTB=np.asarray(cc["smTB"])))
    return in_maps


def kernel(x, src, dst, W0, al0, ar0, b0, W1, al1, ar1, b1, W2, al2, ar2, b2):
    from concourse.bass_utils import run_bass_kernel_spmd

    key = (hash(np.asarray(src).tobytes()) ^ hash(np.asarray(dst).tobytes()))
    if "pre" not in _cache or _cache.get("prekey") != key:
        _cache["pre"] = _preprocess(src, dst)
        _cache["prekey"] = key
    cores, schedA, schedB = _cache["pre"]

    consts = _make_consts(W0, al0, ar0, b0, W1, al1, ar1, b1, W2, al2, ar2, b2)

    ck = key ^ hash(consts["Waug0"].tobytes())
    if "nc" not in _cache or _cache.get("nckey") != ck:
        _cache["nc"] = _build(schedA, schedB, consts)
        _cache["nckey"] = ck
    nc = _cache["nc"]

    in_maps = _in_maps(x)
    r = run_bass_kernel_spmd(nc, in_maps, list(range(NCORE)))
    y = np.zeros((N, D), np.float32)
    for c in range(NCORE):
        lo, hi = _own_rows(c)
        y[lo:hi] = r.results[c]["y"]
    return y


# revision 19
# speedup vs baseline: 1.0466x; 1.0131x over previous
"""3-layer GAT on 8 Trainium2 NeuronCores (Bass/Tile).

Sharding: 2D graph partition. Pair q = cores {2q, 2q+1} aggregates the dst
nodes of strips [q*6250,(q+1)*6250) and [25000+q*6250, 25000+(q+1)*6250);
even cores take edges with src < 25000, odd cores the rest. Node ownership:
core 2k owns rows [k*6250,(k+1)*6250), core 2k+1 owns [25000+k*6250, ...).

Per layer: each core projects its own rows (feat|el|er via an augmented
weight matrix) into a local gather table, then runs the edge phase in two
passes: pass A covers edges whose source is one of the core's own rows and
gathers from the local table while the quad AllGather of the full src-half
table is still in flight; pass B covers the remaining edges and gathers
from the AllGathered table. Edges are packed into 128-wide tiles grouped
by pairs of 128-dst blocks (a tile may straddle the two blocks; the
host-precomputed one-hot masks select membership). Per-edge er comes from
a transposed one-hot matmul against SBUF-resident per-block er rows (no
second gather). Messages are accumulated per dst block by one-hot-mask
matmuls into PSUM; pass B adds pass A's partial sums back in. Partial sums
are pairwise ReduceScattered in two halves (the first fires mid-pass-B),
then a batched divide/bias/ELU (head-mean on the last layer) produces the
output rows, fused with the next layer's projection.
"""

import numpy as np
import ml_dtypes

N = 50000
E = 800000
F = 128                  # input feats and hidden width (4 heads x 32)
H = 4
D = 32
NEG = 0.2
NCORE = 8
NPC = 6250               # nodes owned per core
OWN = 6272               # 49*128, padded own rows
OWNBLK = 49
PAIR = 12544             # 98*128 dst slots per pair
NBLK = 98
NGRP = 49                # pair-groups of 2 blocks
HALF = 25088             # 4*OWN rows per src-half table
TROWS = 25216            # HALF + 128 (dummy row at HALF)
DUMMY = HALF
TCOLS = 256              # bf16 cols: feat(128) | el(4) | pad
CHUNK = 32               # max tiles per dma_gather call
GROUP = 8                # tiles per vector-op batch
EPS = 1e-30

# pair-group processing order: finish half-1 blocks ({0..23} u {49..72})
# first so the first ReduceScatter can fire mid-pass.
GORDER = list(range(0, 12)) + list(range(24, 37)) + \
         list(range(12, 24)) + list(range(37, 49))
H1_BLOCKS = set(range(0, 24)) | set(range(49, 73))

# block -> partial-row-block permutation: [A1(24) B1(24) A2(25) B2(25)]
ROW_OF = np.empty(NBLK, np.int64)
ROW_OF[0:24] = np.arange(24)
ROW_OF[49:73] = 24 + np.arange(24)
ROW_OF[24:49] = 48 + np.arange(25)
ROW_OF[73:98] = 73 + np.arange(25)

_cache = {}


def _schedule(cnt):
    """Core-uniform tile/mask schedule for one pass.

    cnt: [NCORE, NBLK] per-core per-block edge counts.
    Returns dict with T, nmask, chunks, tiles (per tile: list of
    (mslot, block, er_first, er_last, sc_first, sc_last)).
    """
    n0 = cnt[:, 0::2]                      # [NCORE, NGRP]
    n1 = cnt[:, 1::2]
    TP = np.maximum(1, np.ceil((n0 + n1).max(axis=0) / 128).astype(np.int64))

    base_tile = {}
    acc = 0
    for g in GORDER:
        base_tile[g] = acc
        acc += int(TP[g])
    T = acc

    tiles = []            # per tile: list of [mslot, block]
    tile_group = []
    # which (tile-in-group, block-parity) pairs are needed on any core; ensure
    # every block gets at least one occurrence (tile 0 fallback)
    need = {}
    for g in GORDER:
        for i in range(int(TP[g])):
            need[(g, i, 0)] = bool((n0[:, g] > 128 * i).any())
            need[(g, i, 1)] = bool(
                ((n0[:, g] < 128 * (i + 1)) &
                 (n0[:, g] + n1[:, g] > 128 * i)).any())
        if not any(need[(g, i, 0)] for i in range(int(TP[g]))):
            need[(g, 0, 0)] = True
        if not any(need[(g, i, 1)] for i in range(int(TP[g]))):
            need[(g, 0, 1)] = True
    mslot = 0
    for g in GORDER:
        for i in range(int(TP[g])):
            ml = []
            if need[(g, i, 0)]:
                ml.append([mslot, 2 * g])
                mslot += 1
            if need[(g, i, 1)]:
                ml.append([mslot, 2 * g + 1])
                mslot += 1
            assert ml
            tiles.append(ml)
            tile_group.append(g)
    nmask = mslot

    # per-block first/last occurrence
    occ = {}
    for ti, ml in enumerate(tiles):
        for m in ml:
            occ.setdefault(m[1], []).append((ti, m[0]))
    first = {b: o[0] for b, o in occ.items()}
    last = {b: o[-1] for b, o in occ.items()}
    sched_tiles = []
    for ti, ml in enumerate(tiles):
        entry = []
        for k, (ms, b) in enumerate(ml):
            entry.append((ms, b,
                          k == 0, k == len(ml) - 1,
                          first[b] == (ti, ms), last[b] == (ti, ms)))
        sched_tiles.append(entry)

    # chunks aligned to pair-group boundaries, up to CHUNK tiles
    chunks = []
    t0 = 0
    ti = 0
    for g in GORDER:
        ti += int(TP[g])
        nxt = None
        gi = GORDER.index(g)
        if gi + 1 < len(GORDER):
            nxt = int(TP[GORDER[gi + 1]])
        if nxt is None or ti - t0 + nxt > CHUNK:
            m0 = min(m[0] for m in sched_tiles[t0]) if sched_tiles[t0] else 0
            mend = max(m[0] for m in sched_tiles[ti - 1]) + 1
            chunks.append((t0, ti - t0, m0, mend - m0))
            t0 = ti
    assert t0 == T
    return dict(T=T, nmask=nmask, chunks=chunks, tiles=sched_tiles,
                tile_group=tile_group, base_tile=base_tile, TP=TP)


def _wrap16(a):
    # value i of each 128-group at [i%16, i//16], replicated per 16 rows
    t = a.reshape(-1, 128)                     # [T, 128]
    w = t.reshape(t.shape[0], 8, 16)           # [T, 8, 16]
    w = w.transpose(2, 0, 1).reshape(16, -1)   # [16, T*8]
    return np.tile(w, (8, 1)).astype(np.int16)  # [128, T*8]


def _core_pass_arrays(sched, rloc_e, rows_e, pad_row):
    """Build idx + mask streams for one (core, pass).

    rloc_e: pair-local dst row per edge; rows_e: gather-table row per edge.
    """
    T, nmask = sched["T"], sched["nmask"]
    base_tile = sched["base_tile"]
    # group rank of each edge
    grank_of = np.empty(NGRP, np.int64)
    for r, g in enumerate(GORDER):
        grank_of[g] = r
    pg = rloc_e // 256
    gr = grank_of[pg]
    order = np.lexsort((rloc_e, gr))
    rloc_s = rloc_e[order]
    rows_s = rows_e[order]
    gr_s = gr[order]
    # position within group
    starts = np.searchsorted(gr_s, np.arange(len(GORDER)))
    pos_in_group = np.arange(len(gr_s)) - starts[gr_s]
    base128 = np.array([base_tile[GORDER[r]] * 128
                        for r in range(len(GORDER))], np.int64)
    s_glob = base128[gr_s] + pos_in_group

    idx = np.full(T * 128, pad_row, np.int64)
    idx[s_glob] = rows_s

    # mask slot lookup per (tile, block-parity)
    mslot_of = np.full((T, 2), -1, np.int64)
    for ti, ml in enumerate(sched["tiles"]):
        g = sched["tile_group"][ti]
        for (ms, b, *_fl) in ml:
            mslot_of[ti, b - 2 * g] = ms
    ti_e = s_glob // 128
    e_e = s_glob % 128
    b_e = rloc_s // 128
    s128_e = rloc_s % 128
    par = b_e - 2 * np.array(sched["tile_group"])[ti_e]
    ms_e = mslot_of[ti_e, par]
    assert (ms_e >= 0).all()

    smatw = np.zeros((128, nmask * 128), ml_dtypes.bfloat16)
    smatw[e_e, ms_e * 128 + s128_e] = 1
    smatTw = np.zeros((128, nmask * 128), ml_dtypes.bfloat16)
    smatTw[s128_e, ms_e * 128 + e_e] = 1
    return _wrap16(idx), smatw, smatTw


def _preprocess(src, dst):
    src = np.asarray(src).astype(np.int64)
    dst = np.asarray(dst).astype(np.int64)
    q = np.where(dst < 25000, dst // NPC, (dst - 25000) // NPC)
    s = (src >= 25000).astype(np.int64)
    core_of = 2 * q + s
    rloc = np.where(dst < 25000, dst - q * NPC, OWN + (dst - 25000 - q * NPC))
    ks = np.where(src < 25000, src // NPC, (src - 25000) // NPC)
    tloc = np.where(src < 25000, OWN * ks + src - ks * NPC,
                    OWN * ks + (src - 25000) - ks * NPC)
    own = ks == q
    ownrow = np.where(src < 25000, src - ks * NPC, src - 25000 - ks * NPC)
    blk = rloc // 128

    cntA = np.zeros((NCORE, NBLK), np.int64)
    cntB = np.zeros((NCORE, NBLK), np.int64)
    for c in range(NCORE):
        m = core_of == c
        cntA[c] = np.bincount(blk[m & own], minlength=NBLK)
        cntB[c] = np.bincount(blk[m & ~own], minlength=NBLK)
    schedA = _schedule(cntA)
    schedB = _schedule(cntB)

    cores = []
    for c in range(NCORE):
        m = core_of == c
        mA = m & own
        mB = m & ~own
        idxA, smA, smTA = _core_pass_arrays(schedA, rloc[mA], ownrow[mA], OWN)
        idxB, smB, smTB = _core_pass_arrays(schedB, rloc[mB], tloc[mB], DUMMY)
        cores.append(dict(idxA=idxA, smA=smA, smTA=smTA,
                          idxB=idxB, smB=smB, smTB=smTB))
    return cores, schedA, schedB


def _own_rows(c):
    k = c // 2
    if c % 2 == 0:
        return k * NPC, (k + 1) * NPC
    return 25000 + k * NPC, 25000 + (k + 1) * NPC


def _augment(W, al, ar):
    dout = W.shape[1] // H
    Wal = np.stack([W[:, h * dout:(h + 1) * dout] @ al[h] for h in range(H)], 1)
    War = np.stack([W[:, h * dout:(h + 1) * dout] @ ar[h] for h in range(H)], 1)
    return np.concatenate([W, Wal, War], 1).astype(np.float32)  # [128, 136]


def _build(schedA, schedB, consts, no_cc=False):
    import concourse.bass as bass
    import concourse.bacc as bacc
    import concourse.tile as tile
    from concourse import mybir
    from concourse.library_config import mlp

    f32 = mybir.dt.float32
    bf16 = mybir.dt.bfloat16
    i16 = mybir.dt.int16
    AF = mybir.ActivationFunctionType
    OP = mybir.AluOpType

    TA, TB = schedA["T"], schedB["T"]
    NMA, NMB = schedA["nmask"], schedB["nmask"]
    NM_MAX = max(max(nm for (_, _, _, nm) in schedA["chunks"]),
                 max(nm for (_, _, _, nm) in schedB["chunks"]))

    nc = bacc.Bacc(num_devices=NCORE)
    xT_in = nc.declare_dram_parameter("xT", [128, OWN], bf16, isOutput=False)
    idxA_in = nc.declare_dram_parameter("idxA", [128, TA * 8], i16,
                                        isOutput=False)
    idxB_in = nc.declare_dram_parameter("idxB", [128, TB * 8], i16,
                                        isOutput=False)
    smA_in = nc.declare_dram_parameter("smA", [128, NMA * 128], bf16,
                                       isOutput=False)
    smTA_in = nc.declare_dram_parameter("smTA", [128, NMA * 128], bf16,
                                        isOutput=False)
    smB_in = nc.declare_dram_parameter("smB", [128, NMB * 128], bf16,
                                       isOutput=False)
    smTB_in = nc.declare_dram_parameter("smTB", [128, NMB * 128], bf16,
                                        isOutput=False)
    y_out = nc.declare_dram_parameter("y", [NPC, D], f32, isOutput=True)

    with tile.TileContext(nc) as tc:
        with tc.tile_pool(name="persist", bufs=1) as pp, \
             tc.tile_pool(name="dram", bufs=1, space="DRAM") as dp:
            nc.gpsimd.load_library(mlp)

            # ---- persistent SBUF state ----
            idxA_sb = pp.tile([128, TA * 8], i16)
            nc.sync.dma_start(out=idxA_sb[:], in_=idxA_in[:, :])
            idxB_sb = pp.tile([128, TB * 8], i16)
            nc.sync.dma_start(out=idxB_sb[:], in_=idxB_in[:, :])
            hT = pp.tile([128, OWN], bf16)
            nc.sync.dma_start(out=hT[:], in_=xT_in[:, :])
            hT2 = pp.tile([128, OWN], bf16)

            ident_h = nc.inline_tensor(np.eye(128, dtype=np.float32),
                                       name="ident")
            ident_sb = pp.tile([128, 128], f32)
            nc.sync.dma_start(out=ident_sb[:], in_=ident_h[:, :])

            waug_sb = []
            brep_sb = []
            for li in range(3):
                wh = nc.inline_tensor(consts[f"Waug{li}"], name=f"waug{li}")
                wt = pp.tile([128, 136], bf16, name=f"waug_sb{li}")
                nc.sync.dma_start(out=wt[:], in_=wh[:, :])
                waug_sb.append(wt)
                bh = nc.inline_tensor(consts[f"brep{li}"], name=f"brep{li}")
                bt = pp.tile([128, consts[f"brep{li}"].shape[1]], f32,
                             name=f"brep_sb{li}")
                nc.sync.dma_start(out=bt[:], in_=bh[:, :])
                brep_sb.append(bt)

            dummy_h = nc.inline_tensor(consts["dummyrow"], name="dummyrow")

            # ---- DRAM scratch ----
            table = dp.tile([TROWS, TCOLS], bf16)
            er_tab = dp.tile([PAIR, 4], bf16)
            ag_own = dp.tile([OWN + 128, TCOLS], bf16)
            ag_er = dp.tile([OWN, 4], bf16)
            partialA = dp.tile([PAIR, 132], f32)
            partial = dp.tile([PAIR, 132], f32)
            own_sum = dp.tile([OWN, 132], f32)

            nc.sync.dma_start(out=table[DUMMY:DUMMY + 1, :], in_=dummy_h[:, :])
            nc.sync.dma_start(out=ag_own[OWN:OWN + 1, :], in_=dummy_h[:, :])

            groups_pair = [[2 * k, 2 * k + 1] for k in range(4)]
            groups_quad = [[0, 2, 4, 6], [1, 3, 5, 7]]

            def proj_phase(li, src_hT):
                with tc.tile_pool(name=f"prj{li}", bufs=3) as sp, \
                     tc.tile_pool(name=f"prjps{li}", bufs=3,
                                  space="PSUM") as ps:
                    # er-only mini-projection first so the small er AllGather
                    # launches early (pass A consumes it)
                    errow = sp.tile([128, OWNBLK, 4], bf16, name=f"errow{li}",
                                    tag="errow", bufs=1)
                    for t in range(OWNBLK):
                        pje = ps.tile([128, 4], f32, space="PSUM", tag="pje")
                        nc.tensor.matmul(pje[:],
                                         lhsT=src_hT[:, t * 128:(t + 1) * 128],
                                         rhs=waug_sb[li][:, 132:136],
                                         start=True, stop=True)
                        nc.scalar.activation(errow[:, t, :], pje[:], AF.Copy)
                    nc.sync.dma_start(
                        out=ag_er[:, :].rearrange("(t p) c -> p t c", p=128),
                        in_=errow[:])
                    if no_cc:
                        for rep in range(2):
                            nc.sync.dma_start(
                                out=er_tab[rep * OWN:(rep + 1) * OWN, :],
                                in_=ag_er[:, :])
                    else:
                        nc.gpsimd.collective_compute(
                            "AllGather", mybir.AluOpType.bypass,
                            replica_groups=groups_pair,
                            ins=[ag_er[:, :]], outs=[er_tab[:, :]])

                    tabrow = sp.tile([128, OWNBLK, TCOLS], bf16,
                                     name=f"tabrow{li}", tag="tabrow", bufs=1)
                    for t in range(OWNBLK):
                        pj = ps.tile([128, 132], f32, space="PSUM", tag="pj")
                        nc.tensor.matmul(pj[:],
                                         lhsT=src_hT[:, t * 128:(t + 1) * 128],
                                         rhs=waug_sb[li][:, 0:132], start=True,
                                         stop=True)
                        nc.scalar.activation(tabrow[:, t, 0:132], pj[:],
                                             AF.Copy)
                    nc.sync.dma_start(
                        out=ag_own[0:OWN, :]
                            .rearrange("(t p) c -> p t c", p=128),
                        in_=tabrow[:])
                if no_cc:
                    for rep in range(4):
                        nc.sync.dma_start(
                            out=table[rep * OWN:(rep + 1) * OWN, :],
                            in_=ag_own[0:OWN, :])
                else:
                    nc.gpsimd.collective_compute(
                        "AllGather", mybir.AluOpType.bypass,
                        replica_groups=groups_quad,
                        ins=[ag_own[0:OWN, :]], outs=[table[0:HALF, :]])

            proj_phase(0, hT)

            for li in range(3):
                last = li == 2

                with tc.tile_pool(name=f"gt{li}", bufs=4) as gp, \
                     tc.tile_pool(name=f"mk{li}", bufs=2) as mkp, \
                     tc.tile_pool(name=f"ms{li}", bufs=4) as mp, \
                     tc.tile_pool(name=f"ex{li}", bufs=4) as xp, \
                     tc.tile_pool(name=f"pb{li}", bufs=4) as pbp, \
                     tc.tile_pool(name=f"pa{li}", bufs=4) as pap, \
                     tc.tile_pool(name=f"sg{li}", bufs=4,
                                  space="PSUM") as sgps, \
                     tc.tile_pool(name=f"er{li}", bufs=3,
                                  space="PSUM") as erps:
                    er_sb = mp.tile([128, NBLK, 4], bf16, tag="er_sb", bufs=1,
                                    name=f"er_sb{li}")
                    nc.sync.dma_start(
                        out=er_sb[:],
                        in_=er_tab[:, :].rearrange("(t p) c -> p t c", p=128))

                    def edge_pass(sched, idx_sb, sm_in, smT_in, tab, passB):
                        seg_tiles = {}
                        pa_tiles = {}
                        h1rem = [len(H1_BLOCKS)]
                        state = {}

                        def emit_front(ci):
                            # gather + mask streams + er matmuls for chunk ci
                            (t0, nt, m0, nm) = sched["chunks"][ci]
                            g = gp.tile([128, CHUNK, TCOLS], bf16, tag="g")
                            nc.gpsimd.dma_gather(
                                out_ap=g[:, 0:nt, :], in_ap=tab[:, :],
                                idxs_ap=idx_sb[:, t0 * 8:(t0 + nt) * 8],
                                num_idxs=nt * 128, num_idxs_reg=nt * 128,
                                elem_size=TCOLS, single_packet=False)
                            sm = mkp.tile([128, NM_MAX * 128], bf16, tag="sm")
                            nc.sync.dma_start(
                                out=sm[:, 0:nm * 128],
                                in_=sm_in[:, m0 * 128:(m0 + nm) * 128])
                            smT = mkp.tile([128, NM_MAX * 128], bf16,
                                           tag="smT")
                            nc.sync.dma_start(
                                out=smT[:, 0:nm * 128],
                                in_=smT_in[:, m0 * 128:(m0 + nm) * 128])
                            er_ps = erps.tile([128, CHUNK, 4], f32,
                                              space="PSUM", tag="er_ps")
                            for t in range(nt):
                                for (ms, b, ef, el_, _sf, _sl) in \
                                        sched["tiles"][t0 + t]:
                                    lm = ms - m0
                                    nc.tensor.matmul(
                                        er_ps[:, t, :],
                                        lhsT=smT[:, lm * 128:(lm + 1) * 128],
                                        rhs=er_sb[:, b, :],
                                        start=ef, stop=el_)
                            state[ci] = (t0, nt, m0, g, sm, er_ps)

                        def emit_back(ci):
                            (t0, nt, m0, g, sm, er_ps) = state.pop(ci)
                            for g0 in range(0, nt, GROUP):
                                gl = min(GROUP, nt - g0)
                                e4 = xp.tile([128, GROUP, 4], f32, tag="e4")
                                nc.vector.tensor_tensor(
                                    out=e4[:, 0:gl, :],
                                    in0=g[:, g0:g0 + gl, 128:132],
                                    in1=er_ps[:, g0:g0 + gl, :], op=OP.add)
                                lr = xp.tile([128, GROUP, 4], f32, tag="lr")
                                nc.scalar.activation(lr[:, 0:gl, :],
                                                     e4[:, 0:gl, :],
                                                     AF.Prelu, alpha=NEG)
                                ex4 = xp.tile([128, GROUP, 4, 1], f32,
                                              tag="ex4")
                                nc.scalar.activation(ex4[:, 0:gl, :, 0],
                                                     lr[:, 0:gl, :], AF.Exp)
                                m4 = mp.tile([128, GROUP, 132], bf16, tag="m4")
                                nc.scalar.activation(m4[:, 0:gl, 128:132],
                                                     ex4[:, 0:gl, :, 0],
                                                     AF.Copy)
                                nc.vector.tensor_tensor(
                                    out=m4[:, 0:gl, 0:128],
                                    in0=g[:, g0:g0 + gl, 0:128],
                                    in1=ex4[:, 0:gl, :, :]
                                        .to_broadcast([128, gl, 4, 32]),
                                    op=OP.mult)
                                for t in range(gl):
                                    for (ms, b, _ef, _el, sf, sl) in \
                                            sched["tiles"][t0 + g0 + t]:
                                        lm = ms - m0
                                        if sf:
                                            seg_tiles[b] = sgps.tile(
                                                [128, 132], f32, space="PSUM",
                                                tag="seg",
                                                name=f"seg{li}_{passB}_{b}")
                                            if passB:
                                                pa = pap.tile([128, 132], f32,
                                                              tag="pa",
                                                              name=f"pa{li}_{b}")
                                                r = int(ROW_OF[b])
                                                nc.sync.dma_start(
                                                    out=pa[:],
                                                    in_=partialA[
                                                        r * 128:(r + 1) * 128,
                                                        :])
                                                pa_tiles[b] = pa
                                        nc.tensor.matmul(
                                            seg_tiles[b][:],
                                            lhsT=sm[:, lm * 128:(lm + 1) * 128],
                                            rhs=m4[:, t, :],
                                            start=sf, stop=sl)
                                        if sl:
                                            pb = pbp.tile([128, 132], f32,
                                                          tag="pb")
                                            r = int(ROW_OF[b])
                                            if passB:
                                                nc.vector.tensor_tensor(
                                                    out=pb[:],
                                                    in0=seg_tiles[b][:],
                                                    in1=pa_tiles.pop(b)[:],
                                                    op=OP.add)
                                                nc.sync.dma_start(
                                                    out=partial[
                                                        r * 128:(r + 1) * 128,
                                                        :],
                                                    in_=pb[:])
                                                if b in H1_BLOCKS:
                                                    h1rem[0] -= 1
                                                    if h1rem[0] == 0 and \
                                                            not no_cc:
                                                        nc.gpsimd.\
                                                            collective_compute(
                                                            "ReduceScatter",
                                                            mybir.AluOpType.add,
                                                            replica_groups=
                                                            groups_pair,
                                                            ins=[partial[
                                                                0:6144, :]],
                                                            outs=[own_sum[
                                                                0:3072, :]])
                                            else:
                                                nc.scalar.activation(
                                                    pb[:], seg_tiles[b][:],
                                                    AF.Copy)
                                                nc.sync.dma_start(
                                                    out=partialA[
                                                        r * 128:(r + 1) * 128,
                                                        :],
                                                    in_=pb[:])
                                            seg_tiles.pop(b)

                        nchunk = len(sched["chunks"])
                        AHEAD = 1
                        for ci in range(min(AHEAD, nchunk)):
                            emit_front(ci)
                        for ci in range(AHEAD, nchunk):
                            emit_front(ci)
                            emit_back(ci - AHEAD)
                        for ci in range(max(0, nchunk - AHEAD), nchunk):
                            emit_back(ci)

                    edge_pass(schedA, idxA_sb, smA_in, smTA_in, ag_own, False)
                    edge_pass(schedB, idxB_sb, smB_in, smTB_in, table, True)

                if no_cc:
                    nc.sync.dma_start(out=own_sum[0:3072, :],
                                      in_=partial[0:3072, :])
                    nc.sync.dma_start(out=own_sum[3072:OWN, :],
                                      in_=partial[6144:6144 + 3200, :])
                else:
                    nc.gpsimd.collective_compute(
                        "ReduceScatter", mybir.AluOpType.add,
                        replica_groups=groups_pair,
                        ins=[partial[6144:PAIR, :]],
                        outs=[own_sum[3072:OWN, :]])

                # ---- post-processing of own rows (batched) ----
                with tc.tile_pool(name=f"po{li}", bufs=2) as pop, \
                     tc.tile_pool(name=f"pops{li}", bufs=4,
                                  space="PSUM") as tps:
                    osum = pop.tile([128, OWNBLK, 132], f32, name=f"osum{li}",
                                    tag="osum", bufs=1)
                    nc.sync.dma_start(
                        out=osum[:],
                        in_=own_sum[:, :].rearrange("(t p) c -> p t c", p=128))
                    den = pop.tile([128, OWNBLK, 4], f32, tag="den", bufs=1)
                    nc.vector.tensor_scalar_max(den[:], osum[:, :, 128:132],
                                                EPS)
                    rec = pop.tile([128, OWNBLK, 4, 1], f32, tag="rec", bufs=1)
                    nc.vector.reciprocal(rec[:, :, :, 0], den[:])
                    if not last:
                        o2 = pop.tile([128, OWNBLK, 128], f32, tag="o2",
                                      bufs=1)
                        nc.vector.tensor_tensor(
                            out=o2[:].rearrange("p t (h d) -> p t h d", h=4),
                            in0=osum[:, :, 0:128]
                                .rearrange("p t (h d) -> p t h d", h=4),
                            in1=rec[:].to_broadcast([128, OWNBLK, 4, 32]),
                            op=OP.mult)
                        nc.vector.tensor_tensor(
                            out=o2[:], in0=o2[:],
                            in1=brep_sb[li][:]
                                .rearrange("p (t c) -> p t c", t=1)
                                .to_broadcast([128, OWNBLK, 128]),
                            op=OP.add)
                        # ELU via scalar engine: exn = exp(-relu(-x)) =
                        # exp(min(x,0)); o2 = relu(x) + exn - 1
                        exn = pop.tile([128, OWNBLK, 128], f32, tag="exn",
                                       bufs=1)
                        nc.scalar.activation(exn[:], o2[:], AF.Relu,
                                             scale=-1.0)
                        nc.scalar.activation(exn[:], exn[:], AF.Exp,
                                             scale=-1.0)
                        nc.scalar.activation(o2[:], o2[:], AF.Relu)
                        nc.vector.tensor_tensor(out=o2[:], in0=o2[:],
                                                in1=exn[:], op=OP.add)
                        nc.scalar.activation(o2[:], o2[:], AF.Copy, bias=-1.0)
                        dst_hT = hT2 if li % 2 == 0 else hT
                        for b in range(OWNBLK):
                            tp = tps.tile([128, 128], f32, space="PSUM")
                            nc.tensor.matmul(tp[:], lhsT=o2[:, b, :],
                                             rhs=ident_sb[:], start=True,
                                             stop=True)
                            nc.scalar.activation(
                                dst_hT[:, b * 128:(b + 1) * 128],
                                tp[:], AF.Copy)
                    else:
                        r4 = pop.tile([128, OWNBLK, 4, 32], f32, tag="r4",
                                      bufs=1)
                        nc.vector.tensor_tensor(
                            out=r4[:],
                            in0=osum[:, :, 0:128]
                                .rearrange("p t (h d) -> p t h d", h=4),
                            in1=rec[:].to_broadcast([128, OWNBLK, 4, 32]),
                            op=OP.mult)
                        r1 = pop.tile([128, OWNBLK, 32], f32, tag="r1", bufs=1)
                        nc.vector.tensor_tensor(out=r1[:], in0=r4[:, :, 0, :],
                                                in1=r4[:, :, 1, :], op=OP.add)
                        r2 = pop.tile([128, OWNBLK, 32], f32, tag="r2", bufs=1)
                        nc.vector.tensor_tensor(out=r2[:], in0=r4[:, :, 2, :],
                                                in1=r4[:, :, 3, :], op=OP.add)
                        nc.vector.tensor_tensor(out=r1[:], in0=r1[:],
                                                in1=r2[:], op=OP.add)
                        nc.vector.tensor_scalar_mul(r1[:], r1[:], 0.25)
                        nc.vector.tensor_tensor(
                            out=r1[:], in0=r1[:],
                            in1=brep_sb[li][:]
                                .rearrange("p (t c) -> p t c", t=1)
                                .to_broadcast([128, OWNBLK, 32]),
                            op=OP.add)
                        nc.sync.dma_start(
                            out=y_out[0:6144, :]
                                .rearrange("(t p) c -> p t c", p=128),
                            in_=r1[:, 0:48, :])
                        nc.sync.dma_start(
                            out=y_out[6144:NPC, :],
                            in_=r1[0:NPC - 6144, 48, :])

                if not last:
                    proj_phase(li + 1, hT2 if li % 2 == 0 else hT)
    nc.finalize()
    return nc


def _make_consts(W0, al0, ar0, b0, W1, al1, ar1, b1, W2, al2, ar2, b2):
    consts = {}
    for li, (W, al, ar, b) in enumerate(
            [(W0, al0, ar0, b0), (W1, al1, ar1, b1), (W2, al2, ar2, b2)]):
        consts[f"Waug{li}"] = _augment(np.asarray(W, np.float32),
                                       np.asarray(al, np.float32),
                                       np.asarray(ar, np.float32)).astype(
                                           ml_dtypes.bfloat16)
        b = np.asarray(b, np.float32)
        if li < 2:
            consts[f"brep{li}"] = np.tile(b.reshape(1, 128), (128, 1))
        else:
            consts[f"brep{li}"] = np.tile(b.reshape(H, D).mean(0).reshape(1, D),
                                          (128, 1))
    dummy = np.zeros((1, TCOLS), ml_dtypes.bfloat16)
    dummy[0, 128:132] = ml_dtypes.bfloat16(-1e30)
    consts["dummyrow"] = dummy
    return consts


def _in_maps(x):
    cores = _cache["pre"][0]
    x = np.asarray(x, dtype=np.float32)
    in_maps = []
    for c in range(NCORE):
        lo, hi = _own_rows(c)
        xT = np.zeros((128, OWN), ml_dtypes.bfloat16)
        xT[:, 0:NPC] = x[lo:hi].T.astype(ml_dtypes.bfloat16)
        cc = cores[c]
        in_maps.append(dict(xT=xT, idxA=cc["idxA"], idxB=cc["idxB"],
                            smA=np.asarray(cc["smA"]),
                            smTA=np.asarray(cc["smTA"]),
                            smB=np.asarray(cc["smB"]),
                            smTB=np.asarray(cc["smTB"])))
    return in_maps


def kernel(x, src, dst, W0, al0, ar0, b0, W1, al1, ar1, b1, W2, al2, ar2, b2):
    from concourse.bass_utils import run_bass_kernel_spmd

    key = (hash(np.asarray(src).tobytes()) ^ hash(np.asarray(dst).tobytes()))
    if "pre" not in _cache or _cache.get("prekey") != key:
        _cache["pre"] = _preprocess(src, dst)
        _cache["prekey"] = key
    cores, schedA, schedB = _cache["pre"]

    consts = _make_consts(W0, al0, ar0, b0, W1, al1, ar1, b1, W2, al2, ar2, b2)

    ck = key ^ hash(consts["Waug0"].tobytes())
    if "nc" not in _cache or _cache.get("nckey") != ck:
        _cache["nc"] = _build(schedA, schedB, consts)
        _cache["nckey"] = ck
    nc = _cache["nc"]

    in_maps = _in_maps(x)
    r = run_bass_kernel_spmd(nc, in_maps, list(range(NCORE)))
    y = np.zeros((N, D), np.float32)
    for c in range(NCORE):
        lo, hi = _own_rows(c)
        y[lo:hi] = r.results[c]["y"]
    return y
